# revision 29
# baseline (speedup 1.0000x reference)
"""BeamSplitterAttention on 8 TRN2 NeuronCores (Bass/Tile, SPMD).

Decomposition (chosen over the hint's pure options to avoid any cross-core
reduction of the local-stage output):
  Phase A (local stage: nc=4 chunk-wise attention): sharded over the
    chunk-position axis. Each core gets 512 rows of x (64 positions x 2
    batches x 4 chunks) pre-transposed as xaT [d, r]; computes qkv on the
    PE (bf16), the tiny nc=4 softmax-attention on the Vector engine (bf16,
    layouts arranged so every broadcast lands on a non-last axis - both
    operands stay packed 2-byte for the DVE 2x mode), and emits the
    PRE-out-projection attention output o^T: the local out-projection is
    folded into phase B's qkv weights on the host (W = w_in_g @ w_out_l),
    which also removes the out-proj matmuls and weight DMA entirely.
    Each core produces 512 complete, disjoint columns of lattn^T - host
    assembly is a pure gather/permute.
  Phase B (global stage: full attention over S=2048): tensor-parallel over
    the H=16 heads (2 per core). Each core consumes the full lattn^T, its 2
    heads' folded-qkv weight slices, runs scores (PE) -> exp (ACT, fused
    1/sqrt(64) scale, no max-subtraction needed at these scales) -> attn@V
    (PE; a ones-column in the V tiles yields the softmax denominators in
    psum row 64), then a partial out-projection over its 128 concat
    features. Host sums the 8 partial outputs and adds b_out_g.

All matmul operands are bf16 (accumulation fp32 in PSUM); softmax statistics
are fp32 where cheap. Each block's normalize+out-proj tail is emitted inside
the next block's score stream so the in-order PE queue never stalls on the
ACT/DVE chain.
"""
import numpy as np
import ml_dtypes

import concourse.bass as bass
import concourse.tile as tile
from concourse import bacc, mybir
from concourse.bass_utils import run_bass_kernel_spmd
from concourse.masks import make_identity

F32 = mybir.dt.float32
F32R = mybir.dt.float32r
BF = mybir.dt.bfloat16
NPBF = ml_dtypes.bfloat16
AX = mybir.AxisListType
OP = mybir.AluOpType
ACTF = mybir.ActivationFunctionType
HINTS = (mybir.EngineType.PE, mybir.EngineType.Activation,
         mybir.EngineType.DVE, mybir.EngineType.SP)

P = 128
D = 1024
H = 16
HD = 64
B = 2
S = 2048
NCH = 4
NCORES = 8
ROWS_A = 512
RT_B = 8

# phase A head groups (start head, head count); boundaries must be multiples
# of 2 heads = 128 features for the transposes. Even quarters measured best
# (bigger first group delays pipeline fill; smaller tail groups waste DVE
# op efficiency).
A_GROUPS = [(0, 4), (4, 4), (8, 4), (12, 4)]

ABLATE = set()


def _loop(tc, loop_r):
    import contextlib
    if loop_r:
        return tc.For_i(0, loop_r, 1, hint_engines=HINTS,
                        staggered_reset=True)
    return contextlib.nullcontext()


# ---------------------------- phase A ----------------------------

def build_phase_a(loop_r=None, unroll=1):
    nc = bacc.Bacc("TRN2", target_bir_lowering=False, debug=False,
                   num_devices=NCORES)
    xaT = nc.dram_tensor("xaT", (D, ROWS_A), BF, kind="ExternalInput").ap()
    winlT = nc.dram_tensor("winlT", (D, 3 * D), BF, kind="ExternalInput").ap()
    binl = nc.dram_tensor("binl", (3 * D,), F32R, kind="ExternalInput").ap()
    locTa = nc.dram_tensor("locTa", (D, ROWS_A), BF, kind="ExternalOutput").ap()

    with tile.TileContext(nc) as tc:
        with nc.allow_low_precision(reason="bf16 kernel by design"):
            _phase_a_body(tc, xaT, winlT, binl, locTa, loop_r, unroll)
    nc.compile()
    return nc


def _phase_a_body(tc, xaT, winlT, binl, locTa, loop_r=None, unroll=1):
    nc = tc.nc
    KT = 8

    import contextlib
    ctx = contextlib.ExitStack()
    const = ctx.enter_context(tc.tile_pool(name="const", bufs=1))
    wpool = ctx.enter_context(tc.tile_pool(name="w", bufs=2))
    tpool = ctx.enter_context(tc.tile_pool(name="tmp", bufs=2))
    apool = ctx.enter_context(tc.tile_pool(name="attn", bufs=1))
    psq = ctx.enter_context(tc.tile_pool(name="psq", bufs=6, space="PSUM"))
    pst = ctx.enter_context(tc.tile_pool(name="pst", bufs=2, space="PSUM"))

    xa = const.tile([P, KT, ROWS_A], BF)
    nc.sync.dma_start(xa[:], xaT.rearrange("(kt p) r -> p kt r", p=P))
    bias_sb = const.tile([1, 3 * D], F32R)
    nc.sync.dma_start(bias_sb[:], binl.rearrange("(a f) -> a f", a=1))
    ones_r = const.tile([1, P], F32R)
    ones_f32a = const.tile([1, P], F32)
    nc.vector.memset(ones_f32a[:], 1.0)
    nc.vector.tensor_copy(ones_r[:], ones_f32a[:])
    ident = const.tile([P, P], BF)
    make_identity(nc, ident)

    qkv = const.tile([P, NCH, 3 * D], BF)
    o_sb = const.tile([P, NCH, D], BF)
    oT = const.tile([P, KT, ROWS_A], BF)

    with _loop(tc, loop_r):
        for ui in range(unroll):
            _phase_a_compute(tc, locTa, winlT, xa, bias_sb, ones_r, ident,
                             qkv, o_sb, oT, wpool, tpool, apool, psq, pst,
                             ui=ui)
    ctx.close()


def _phase_a_compute(tc, locTa, winlT, xa, bias_sb, ones_r, ident,
                     qkv, o_sb, oT, wpool, tpool, apool, psq, pst, ui=0):
    nc = tc.nc
    KT = 8
    QG = len(A_GROUPS)

    if "a_empty" in ABLATE:
        return

    def make_tr(qq):
        # transpose group qq of o -> oT (bf16 PE transpose); emitted one
        # group late so the PE transposes overlap the next group's DVE
        # attention instead of waiting on it.
        h0, hc = A_GROUPS[qq]
        f0, f1 = (h0 * HD) // P, ((h0 + hc) * HD) // P

        def tr():
            for cq in range(NCH):
                for ftl in range(f0, f1):
                    tps = pst.tile([P, P], BF, tag="tp", name=f"tp_{ui}_{qq}_{cq}_{ftl}")
                    nc.tensor.transpose(tps[:], o_sb[:, cq, ftl * P:(ftl + 1) * P],
                                        ident[:])
                    nc.scalar.copy(out=oT[:, ftl, cq * P:(cq + 1) * P], in_=tps[:])
            for ftl in range(f0, f1):
                nc.sync.dma_start(locTa[ftl * P:(ftl + 1) * P, :], oT[:, ftl, :])
        return tr

    def qkv_sec(qq, sec):
        # one projection section (q, k or v) of one head group
        h0, hc = A_GROUPS[qq]
        off, gw = sec * D + h0 * HD, hc * HD
        wts = wpool.tile([P, KT, gw], BF, tag="winl", name=f"w_{ui}_{qq}_{sec}")
        for h in range(2):
            nc.sync.dma_start(
                wts[:, 4 * h:4 * h + 4, :],
                winlT[512 * h:512 * h + 512,
                      off:off + gw].rearrange("(kt p) f -> p kt f", p=P),
            )
        for c in range(NCH):
            ps = psq.tile([P, gw], F32, tag="mm", name=f"ps_{ui}_{qq}_{sec}_{c}")
            for kt in range(KT):
                nc.tensor.matmul(ps[:], xa[:, kt, c * P:(c + 1) * P],
                                 wts[:, kt, :], start=(kt == 0), stop=False)
            nc.tensor.matmul(ps[:], ones_r[0:1, 0:P],
                             bias_sb[0:1, off:off + gw],
                             start=False, stop=True)
            nc.scalar.copy(out=qkv[:, c, off:off + gw], in_=ps[:])

    pending_tr = None
    for qq in range(QG):
        # q and k sections first: the DVE scores for this quarter overlap
        # the v-section matmuls on the PE.
        qkv_sec(qq, 0)
        if pending_tr is not None:
            pending_tr()
            pending_tr = None
        qkv_sec(qq, 1)

        if "a_no_attn" in ABLATE:
            qkv_sec(qq, 2)
            continue
        # local attention over chunks (nc=4) on DVE, this group.
        # q/k sections are h-major (h, d); the v section is d-major (d, h)
        # via a host-side column permutation of winlT, so that every DVE
        # broadcast below lands on a non-last axis (2x mode stays eligible).
        h0, hc = A_GROUPS[qq]
        off, gw = h0 * HD, hc * HD
        qv = qkv[:, :, off: off + gw].rearrange("p c (h d) -> p c h d", h=hc)
        kv = qkv[:, :, D + off: D + off + gw].rearrange(
            "p c (h d) -> p c h d", h=hc)
        vv = qkv[:, :, 2 * D + off: 2 * D + off + gw].rearrange(
            "p c (d h) -> p c d h", h=hc)

        # scores s[c, c', h]
        s_t = apool.tile([P, NCH, NCH, hc], BF, tag="s", name=f"s_{ui}_{qq}")
        for ck in range(NCH):
            tmp = tpool.tile([P, NCH, hc, HD], BF, tag="tmp", name=f"tmp_{ui}_{qq}_{ck}")
            kb = kv[:, ck][:, None].to_broadcast((P, NCH, hc, HD))
            nc.vector.tensor_tensor(tmp[:], qv[:], kb, OP.mult)
            nc.vector.reduce_sum(s_t[:, :, ck, :], tmp[:], axis=AX.X)

        e_t = apool.tile([P, NCH, NCH, hc], BF, tag="e", name=f"e_{ui}_{qq}")
        nc.scalar.activation(e_t[:], s_t[:], ACTF.Exp, scale=1.0 / np.sqrt(HD))
        den = apool.tile([P, NCH, hc], F32, tag="den", name=f"den_{ui}_{qq}")
        nc.vector.reduce_sum(den[:], e_t.rearrange("p c k h -> p c h k"),
                             axis=AX.X)
        denb = apool.tile([P, NCH, hc], BF, tag="denb", name=f"denb_{ui}_{qq}")
        nc.vector.reciprocal(denb[:], den[:])
        nc.vector.tensor_tensor(
            e_t[:], e_t[:],
            denb[:, :, None, :].to_broadcast((P, NCH, NCH, hc)), OP.mult)

        # v section on the PE while the DVE runs the scores above
        qkv_sec(qq, 2)

        # o[c, d, h] = sum_c' e[c, c', h] * v[c', d, h]
        ov = o_sb[:, :, off: off + gw].rearrange("p c (d h) -> p c d h", h=hc)
        for ck in range(NCH):
            eb = e_t[:, :, ck, :][:, :, None, :].to_broadcast((P, NCH, HD, hc))
            vb = vv[:, ck][:, None].to_broadcast((P, NCH, HD, hc))
            if ck == 0:
                nc.vector.tensor_tensor(ov[:], eb, vb, OP.mult)
            else:
                tmp = tpool.tile([P, NCH, HD, hc], BF, tag="tmp",
                                 name=f"tmpo_{ui}_{qq}_{ck}")
                nc.vector.tensor_tensor(tmp[:], eb, vb, OP.mult)
                nc.vector.tensor_tensor(ov[:], ov[:], tmp[:], OP.add)

        if "a_no_tr" in ABLATE:
            continue
        pending_tr = make_tr(qq)

    if pending_tr is not None:
        pending_tr()


# ---------------------------- phase B ----------------------------

def build_phase_b(loop_r=None, unroll=1):
    nc = bacc.Bacc("TRN2", target_bir_lowering=False, debug=False,
                   num_devices=NCORES)
    locT = nc.dram_tensor("locT", (D, B * S), BF, kind="ExternalInput").ap()
    wqT = nc.dram_tensor("wqT", (D, P), BF, kind="ExternalInput").ap()
    wkT = nc.dram_tensor("wkT", (D, P), BF, kind="ExternalInput").ap()
    wvT = nc.dram_tensor("wvT", (D, P), BF, kind="ExternalInput").ap()
    bq = nc.dram_tensor("bq", (P,), F32, kind="ExternalInput").ap()
    bk = nc.dram_tensor("bk", (P,), F32, kind="ExternalInput").ap()
    bv = nc.dram_tensor("bv", (P,), F32, kind="ExternalInput").ap()
    woT = nc.dram_tensor("woT", (P, D), BF, kind="ExternalInput").ap()
    outTp = nc.dram_tensor("outTp", (D, B * S), BF, kind="ExternalOutput").ap()

    with tile.TileContext(nc) as tc:
        with nc.allow_low_precision(reason="bf16 kernel by design"):
            _phase_b_body(tc, locT, wqT, wkT, wvT, bq, bk, bv, woT, outTp,
                          loop_r, unroll)
    nc.compile()
    return nc


def _phase_b_body(tc, locT, wqT, wkT, wvT, bq, bk, bv, woT, outTp,
                  loop_r=None, unroll=1):
    nc = tc.nc
    KT = 8

    import contextlib
    ctx = contextlib.ExitStack()
    const = ctx.enter_context(tc.tile_pool(name="const", bufs=1))
    lpool = ctx.enter_context(tc.tile_pool(name="loc", bufs=3))
    epool = ctx.enter_context(tc.tile_pool(name="exp", bufs=6))
    apool = ctx.enter_context(tc.tile_pool(name="attn", bufs=2))
    dpool = ctx.enter_context(tc.tile_pool(name="den", bufs=2))
    outp = ctx.enter_context(tc.tile_pool(name="out", bufs=3))
    vt_pool = ctx.enter_context(tc.tile_pool(name="vt", bufs=2))
    ps_big = ctx.enter_context(tc.tile_pool(name="ps_big", bufs=2, space="PSUM"))
    ps_med = ctx.enter_context(tc.tile_pool(name="ps_med", bufs=2, space="PSUM"))
    ps_acc = ctx.enter_context(tc.tile_pool(name="ps_acc", bufs=2, space="PSUM"))

    wq_t = const.tile([P, KT, P], BF)
    nc.sync.dma_start(wq_t[:], wqT.rearrange("(kt p) f -> p kt f", p=P))
    wk_t = const.tile([P, KT, P], BF)
    nc.sync.dma_start(wk_t[:], wkT.rearrange("(kt p) f -> p kt f", p=P))
    wv_t = const.tile([P, KT, P], BF)
    nc.sync.dma_start(wv_t[:], wvT.rearrange("(kt p) f -> p kt f", p=P))
    wo_t = const.tile([P, D], BF)
    nc.sync.dma_start(wo_t[:], woT[:, :])
    bq_t = const.tile([P, 1], F32)
    nc.sync.dma_start(bq_t[:], bq.rearrange("(o p) -> p o", p=P))
    bk_t = const.tile([P, 1], F32)
    nc.sync.dma_start(bk_t[:], bk.rearrange("(o p) -> p o", p=P))
    bv_t = const.tile([P, 1], F32)
    nc.sync.dma_start(bv_t[:], bv.rearrange("(o p) -> p o", p=P))
    ident = const.tile([P, P], BF)
    make_identity(nc, ident)
    ones_f = const.tile([P, HD], BF)
    nc.vector.memset(ones_f[:], 1.0)

    qT = const.tile([P, B * S], BF)
    kT = const.tile([P, B * S], BF)
    v_sb = const.tile([P, 32, 130], BF)
    nc.vector.tensor_copy(
        v_sb.rearrange("p r (j f) -> p r j f", f=65)[:, :, :, 64],
        ones_f.rearrange("p (a b) -> p a b", b=2),
    )

    with _loop(tc, loop_r):
        for ui in range(unroll):
            _phase_b_compute(tc, locT, outTp, wq_t, wk_t, wv_t, wo_t,
                             bq_t, bk_t, bv_t, ident,
                             qT, kT, v_sb, lpool, epool, apool, dpool, outp,
                             vt_pool, ps_big, ps_med, ps_acc, ui=ui)
    ctx.close()


def _phase_b_compute(tc, locT, outTp, wq_t, wk_t, wv_t, wo_t,
                     bq_t, bk_t, bv_t, ident,
                     qT, kT, v_sb, lpool, epool, apool, dpool, outp,
                     vt_pool, ps_big, ps_med, ps_acc, ui=0):
    nc = tc.nc
    KT = 8
    NKT = 16
    NQT = 4
    LAG = 4

    # stage 1: q/k/v projections + v transpose. The QK and V matmul streams
    # are split so each rt's QK-psum drain (DVE bias-adds) overlaps the V
    # matmuls; V-transposes are pipelined one rt behind so they never wait
    # on the freshly-written vT_s.
    def make_vtr(rt, vT_s):
        def vtr():
            for i in range(4):
                tps = ps_med.tile([P, P], BF, tag="m")
                nc.tensor.transpose(tps[:], vT_s[:, i * P:(i + 1) * P], ident[:])
                nc.vector.tensor_copy(
                    v_sb[:, rt * 4 + i, :].rearrange("p (j f) -> p j f", j=2)[:, :, 0:64],
                    tps[:].rearrange("p (j f) -> p j f", j=2),
                )
        return vtr

    pending_vtr = None
    for rt in range(RT_B):
        rsl = slice(rt * 512, (rt + 1) * 512)
        loc_t = lpool.tile([P, KT, 512], BF, tag="loc")
        for h in range(2):
            nc.sync.dma_start(
                loc_t[:, 4 * h:4 * h + 4, :],
                locT[512 * h:512 * h + 512, rsl].rearrange("(kt p) r -> p kt r", p=P))
        ps_qk = ps_big.tile([P, 1024], F32, tag="sps")
        for kt in range(KT):
            st, sp = kt == 0, kt == KT - 1
            nc.tensor.matmul(ps_qk[:, 0:512], wq_t[:, kt, :], loc_t[:, kt, :], start=st, stop=sp)
            nc.tensor.matmul(ps_qk[:, 512:1024], wk_t[:, kt, :], loc_t[:, kt, :], start=st, stop=sp)
        if pending_vtr is not None:
            pending_vtr()
            pending_vtr = None
        ps_v = ps_big.tile([P, 1024], F32, tag="sps")
        for kt in range(KT):
            nc.tensor.matmul(ps_v[:, 0:512], wv_t[:, kt, :], loc_t[:, kt, :],
                             start=(kt == 0), stop=(kt == KT - 1))
        nc.vector.tensor_scalar_add(qT[:, rsl], ps_qk[:, 0:512], bq_t[:, 0:1])
        nc.vector.tensor_scalar_add(kT[:, rsl], ps_qk[:, 512:1024], bk_t[:, 0:1])
        vT_s = vt_pool.tile([P, 512], BF, tag="vts")
        nc.vector.tensor_scalar_add(vT_s[:], ps_v[:, 0:512], bv_t[:, 0:1])
        pending_vtr = make_vtr(rt, vT_s)
    pending_vtr()

    if "stage1_only" in ABLATE:
        return

    # stage 2: attention, with each block's normalize+out-proj tail emitted
    # inside the NEXT block's score stream (keeps the in-order PE queue fed).
    def make_tail(o_ps, dens, attn, qsl):
        def tail():
            for j in range(2):
                bc_sb = dpool.tile([64, 512], F32, tag="bcsb")
                nc.gpsimd.partition_broadcast(bc_sb[:], dens[j][0:1, :])
                nc.vector.tensor_tensor(attn[64 * j:64 * j + 64, :],
                                        o_ps[j][0:64, :], bc_sb[:], OP.mult)
            for dt in range(KT):
                ps = ps_med.tile([P, 512], F32, tag="m")
                nc.tensor.matmul(ps[:], wo_t[:, dt * P:(dt + 1) * P], attn[:],
                                 start=True, stop=True)
                ot = outp.tile([P, 512], BF, tag="out")
                nc.vector.tensor_copy(ot[:], ps[:])
                nc.sync.dma_start(outTp[dt * P:(dt + 1) * P, qsl], ot[:])
        return tail

    pending_tail = None
    for b in range(B):
        for qt in range(NQT):
            qsl = slice(b * S + qt * 512, b * S + (qt + 1) * 512)
            attn = apool.tile([P, 512], BF, tag="attn", name=f"attn{ui}_{b}_{qt}")
            o_ps = None
            e_ts = {}

            def do_av(kt, b=b, qt=qt, e_ts=e_ts):
                e_kt = e_ts.pop(kt)
                for j in range(2):
                    nc.tensor.matmul(o_ps[j][:],
                                     v_sb[:, b * 16 + kt, 65 * j:65 * j + 65],
                                     e_kt[:, j * 512:(j + 1) * 512],
                                     start=(kt == 0), stop=(kt == NKT - 1))

            for kt in range(NKT):
                ksl = slice(b * S + kt * P, b * S + (kt + 1) * P)
                sps = ps_big.tile([P, 1024], F32, tag="sps")
                e_t = epool.tile([P, 1024], BF, tag="et", name=f"e_t{ui}_{b}_{qt}_{kt}")
                for j in range(2):
                    fsl = slice(64 * j, 64 * j + 64)
                    nc.tensor.matmul(sps[:, j * 512:(j + 1) * 512],
                                     kT[fsl, ksl], qT[fsl, qsl],
                                     start=True, stop=True)
                if "no_exp" not in ABLATE:
                    nc.scalar.activation(e_t[:], sps[:], ACTF.Exp, scale=1.0 / np.sqrt(HD))
                else:
                    nc.vector.tensor_copy(e_t[:, 0:8], sps[:, 0:8])
                e_ts[kt] = e_t
                if kt == LAG - 1 and pending_tail is not None:
                    pending_tail()
                    pending_tail = None
                if kt == LAG - 1:
                    o_ps = [ps_acc.tile([65, 512], F32, tag="o",
                                        name=f"o_ps{ui}_{b}_{qt}_{j}") for j in range(2)]
                if kt >= LAG:
                    do_av(kt - LAG)
            for kt in range(NKT - LAG, NKT):
                do_av(kt)

            if "no_tail" in ABLATE:
                continue
            # reciprocals start as soon as the denominators land in psum
            # row 64; only the broadcast/normalize/out-proj is deferred.
            dens = []
            for j in range(2):
                den = dpool.tile([1, 512], F32, tag="den",
                                 name=f"den{ui}_{b}_{qt}_{j}")
                nc.vector.reciprocal(den[0:1, :], o_ps[j][64:65, :])
                dens.append(den)
            pending_tail = make_tail(o_ps, dens, attn, qsl)
    if pending_tail is not None:
        pending_tail()


# ---------------- host-side prep / assembly ----------------

def _bf(x):
    return np.ascontiguousarray(np.asarray(x, np.float32).astype(NPBF))


def _perm_feat():
    """Phase A emits lattn^T rows f = off_g + d*hc + hl per head group;
    map to the natural head-space feature (h0_g + hl)*64 + d."""
    perm = np.empty(D, np.int64)
    for h0, hc in A_GROUPS:
        off, gw = h0 * HD, hc * HD
        r = np.arange(gw)
        d, hl = r // hc, r % hc
        perm[off:off + gw] = (h0 + hl) * HD + d
    return perm


def _perm_winl_cols():
    """Column order for winlT (= row order of w_in_l): q/k groups natural
    (h-major), v section d-major within each head group."""
    idx = np.arange(3 * D)
    out = idx.copy()
    for h0, hc in A_GROUPS:
        off, gw = h0 * HD, hc * HD
        r = np.arange(gw)
        d, hl = r % HD, r // HD          # natural within-group: hl*64 + d
        out[2 * D + off: 2 * D + off + gw] = 2 * D + off + d * hc + hl
    # out[pos] gives the DESTINATION column for natural row pos; invert:
    inv = np.empty_like(out)
    inv[out] = idx
    return inv  # winlT[:, j] = w_in_l.T[:, inv[j]]


def prep_phase_a_inputs(x, w_in_l, b_in_l):
    x = np.asarray(x, np.float32)
    xr = x.reshape(B, NCH, NCORES, 64, D)
    xa = np.transpose(xr, (2, 1, 0, 3, 4)).reshape(NCORES, ROWS_A, D)
    inv = _perm_winl_cols()
    winlT = _bf(np.asarray(w_in_l, np.float32).T[:, inv])
    binl = np.ascontiguousarray(np.asarray(b_in_l, np.float32)[inv])
    in_maps = []
    for k in range(NCORES):
        in_maps.append({
            "xaT": _bf(xa[k].T),
            "winlT": winlT,
            "binl": binl,
        })
    return in_maps


def assemble_locT(results):
    A = np.stack([np.asarray(results[k]["locTa"]) for k in range(NCORES)])
    locT = (A.reshape(NCORES, D, NCH, B, 64)
             .transpose(1, 2, 3, 0, 4)
             .reshape(D, B * S))
    return np.ascontiguousarray(locT)


def prep_phase_b_inputs(locT, w_out_l, b_out_l, w_in_g, b_in_g, w_out_g):
    w_out_l = np.asarray(w_out_l, np.float32)
    b_out_l = np.asarray(b_out_l, np.float32)
    w_in_g = np.asarray(w_in_g, np.float32)
    b_in_g = np.asarray(b_in_g, np.float32)
    w_out_g = np.asarray(w_out_g, np.float32)
    # fold the local out-projection into the global in-projection
    W = w_in_g @ w_out_l              # (3D, D), columns in head-space
    bm = w_in_g @ b_out_l + b_in_g    # (3D,)
    perm = _perm_feat()
    W = W[:, perm]                    # match phase A's emitted row order
    in_maps = []
    for k in range(NCORES):
        sl = slice(128 * k, 128 * k + 128)
        in_maps.append({
            "locT": locT,
            "wqT": _bf(W[sl, :].T),
            "wkT": _bf(W[D + 128 * k: D + 128 * k + 128, :].T),
            "wvT": _bf(W[2 * D + 128 * k: 2 * D + 128 * k + 128, :].T),
            "bq": np.ascontiguousarray(bm[sl]),
            "bk": np.ascontiguousarray(bm[D + 128 * k: D + 128 * k + 128]),
            "bv": np.ascontiguousarray(bm[2 * D + 128 * k: 2 * D + 128 * k + 128]),
            "woT": _bf(w_out_g[:, sl].T),
        })
    return in_maps


def assemble_output(results, b_out_g):
    outT = np.sum([np.asarray(results[k]["outTp"]).astype(np.float32)
                   for k in range(NCORES)], axis=0)
    outT += np.asarray(b_out_g, np.float32)[:, None]
    return np.ascontiguousarray(outT.T.reshape(B, S, D))


_CACHE = {}


def kernel(x, w_in_l, b_in_l, w_out_l, b_out_l, w_in_g, b_in_g, w_out_g, b_out_g):
    if "a" not in _CACHE:
        _CACHE["a"] = build_phase_a()
    if "b" not in _CACHE:
        _CACHE["b"] = build_phase_b()
    core_ids = list(range(NCORES))
    in_a = prep_phase_a_inputs(x, w_in_l, b_in_l)
    res_a = run_bass_kernel_spmd(_CACHE["a"], in_a, core_ids=core_ids)
    locT = assemble_locT(res_a.results)
    in_b = prep_phase_b_inputs(locT, w_out_l, b_out_l, w_in_g, b_in_g, w_out_g)
    res_b = run_bass_kernel_spmd(_CACHE["b"], in_b, core_ids=core_ids)
    return assemble_output(res_b.results, b_out_g)
